# revision 25
# baseline (speedup 1.0000x reference)
"""Trainium2 Bass kernel for nn_L2PppMaskAttn (topk_masking).

Math reformulation of the reference:
  - top-5 ranking over prompts is invariant to q normalization, so scores
    u[b,p] = <x[b,l], K_hat[l,p]> suffice; mask = (u >= 5th_max(u)).
  - a_k depends only on (layer, prompt): s[l,p] = <K_hat[l,p], A_hat[l,p]>.
  - out[l,b] = (mask_row .* s) @ P_flat[l]: a [128,100] @ [100,6144] matmul.

Numerical contract: everything feeding the top-5 SELECTION replicates the
known-good op sequence bit-for-bit (scalar Square+accum for ||K||^2, sqrt,
reciprocal + one Newton step, f32 elementwise K*rinv products, PE f32
matmuls in the same 6x128-chunk accumulation order).  The tightest 5th/6th
score gap in this input set is ~1e-6 and a single flipped selection costs
~0.2 rel error, so this path must not be re-ordered.  Batching layers into
[100,k] tiles keeps ops elementwise-identical, so it stays bit-exact.  The
top-5 threshold comes from DVE max8 (comparison-only, exact).  The
output-scale path (s, P matmul, store) runs in bf16: ~5e-3 worst case vs
the 2e-2 gate.

Schedule: a prelude computes rinv and its row-broadcast for ALL layers (3
PE matmuls); phase 1 runs per-layer selection -> W^T (nkt, 6 f32 score
matmuls, max8 threshold, one bf16 gram for s, mask transpose); phase 2 is
a pure streaming loop (12 bf16 matmuls + PSUM casts + one 1.5 MB store per
layer).  Inputs land as a 2-layer head plus two ~5 MB bulk halves so layer
0 starts within ~5us and loads never starve mid-kernel.  P loads ride the
gpsimd ring, stores the scalar ring, everything else the sync ring.

Host-packed layouts (no device transposes, no device casts):
  x^T   [128dd, (l,j,b)]        f32   4.7 MB
  K^T   [128dd, (l,j,p)]        f32   3.7 MB   (scores)
  K     [100p, (l,d)]           f32   3.7 MB   (||K||^2, exact path)
  KA^T  [128dd, (l,j,[K|A])]    bf16  3.7 MB   (s-gram)
  P     [l][100p, 6144]         bf16 14.8 MB
  out   [l][128b, 6144]         bf16 18.9 MB
~49.5 MB HBM traffic per core vs ~79 MB for the f32 baseline.

Sharding: data-parallel over batch, 8 cores x 128 rows; K/A/P replicated.
"""

import sys

sys.path.insert(0, "/opt/trn_rl_repo")

import numpy as np

B, L, P_N, LP, D = 1024, 12, 100, 8, 768
N_CORES = 8
BS = B // N_CORES  # 128 batch rows per core
NF = LP * D  # 6144 flattened output features per layer
NCH = D // 128  # 6 contraction chunks
C = NCH * P_N  # 600 K^T columns per layer
C2 = 2 * C  # interleaved [K|A] columns per layer
TOP_K = 5
LA = 2  # layers in the early head loads

_CACHE = {}


def _build_nc():
    if "nc" in _CACHE:
        return _CACHE["nc"]

    from contextlib import ExitStack

    import concourse.bass as bass
    import concourse.bacc as bacc
    import concourse.mybir as mybir
    from concourse import masks
    from concourse.tile import TileContext

    f32 = mybir.dt.float32
    bf16 = mybir.dt.bfloat16
    AX = mybir.AxisListType
    OP = mybir.AluOpType
    AF = mybir.ActivationFunctionType

    nc = bacc.Bacc(
        "TRN2",
        target_bir_lowering=False,
        debug=False,
        num_devices=N_CORES,
    )

    xt_d = nc.declare_dram_parameter("x", [128, L * D], f32, isOutput=False)
    kt_d = nc.declare_dram_parameter("kt", [128, L * C], f32, isOutput=False)
    ka_d = nc.declare_dram_parameter("ka", [128, L * C2], bf16, isOutput=False)
    kn_d = nc.declare_dram_parameter("kn", [P_N, L * D], f32, isOutput=False)
    p_d = nc.declare_dram_parameter("p", [L, P_N, NF], bf16, isOutput=False)
    o_d = nc.declare_dram_parameter("o", [L, BS, NF], bf16, isOutput=True)

    with TileContext(nc) as tc, ExitStack() as ctx:
        pool = lambda name, bufs, **kw: ctx.enter_context(
            tc.tile_pool(name=name, bufs=bufs, **kw)
        )
        const = pool("const", 1)
        ppool = pool("pp", 2)
        nktp = pool("nktp", 2)
        scrp = pool("scrp", 1)
        rowp = pool("rowp", 2)
        small = pool("small", 2)
        obuf = pool("ob", 2)
        ps_pc = pool("ps_pc", 1, space="PSUM")
        ps_rb = pool("ps_rb", 1, space="PSUM")
        ps_g = pool("ps_g", 1, space="PSUM")
        ps_mt = pool("ps_mt", 1, space="PSUM")
        ps_o = pool("ps_o", 2, space="PSUM")

        ident = const.tile([128, 128], f32, tag="ident")
        masks.make_identity(nc, ident[:])
        ones_col = const.tile([100, 128], f32, tag="ones")
        nc.vector.memset(ones_col[:], 1.0)
        # [I | I] for extracting both gram diagonals in one pass
        ident2 = const.tile([P_N, 2 * P_N], f32, tag="ident2")
        nc.gpsimd.tensor_copy(ident2[:, :P_N], ident[:P_N, :P_N])
        nc.gpsimd.tensor_copy(ident2[:, P_N:], ident[:P_N, :P_N])

        wt_all = const.tile([P_N, L * BS], bf16, tag="wt")
        rb_all = const.tile([128, L * P_N], f32, tag="rb")

        # ---- input loads: 2-layer head, then two bulk halves ----
        kn_all = const.tile([P_N, L * D], f32, tag="kn")
        kt_all = const.tile([128, L * C], f32, tag="kt")
        xt_all = const.tile([128, L * D], f32, tag="xt")
        ka_all = const.tile([128, L * C2], bf16, tag="ka")

        # Head loads (layers 0..LA-1) go first and ALONE: the bulk loads are
        # gated behind marker copies that depend on head data, otherwise the
        # DMA engines round-robin the bulk alongside the heads and the
        # critical path waits ~20us for 600 KB.  kn head rides the (idle)
        # scalar ring so it is not queued behind the other heads.
        nc.scalar.dma_start(kn_all[:, : LA * D], kn_d[:, : LA * D])
        nc.sync.dma_start(kt_all[:, : LA * C], kt_d[:, : LA * C])
        nc.sync.dma_start(xt_all[:, : LA * D], xt_d[:, : LA * D])
        nc.sync.dma_start(ka_all[:, : LA * C2], ka_d[:, : LA * C2])

        def _gate(dst_ap, src_ap):
            # 1-element copy: reads head-loaded data, writes the bulk
            # region -> the bulk DMA (WAW) waits until the head landed.
            nc.gpsimd.tensor_copy(dst_ap, src_ap)

        _gate(kn_all[:1, LA * D : LA * D + 1], kn_all[:1, :1])
        _gate(kt_all[:1, LA * C : LA * C + 1], kt_all[:1, :1])
        _gate(xt_all[:1, LA * D : LA * D + 1], xt_all[:1, :1])
        _gate(ka_all[:1, LA * C2 : LA * C2 + 1], ka_all[:1, :1])

        def _loads(l0, l1):
            nc.sync.dma_start(kn_all[:, l0 * D : l1 * D], kn_d[:, l0 * D : l1 * D])
            nc.sync.dma_start(kt_all[:, l0 * C : l1 * C], kt_d[:, l0 * C : l1 * C])
            nc.sync.dma_start(xt_all[:, l0 * D : l1 * D], xt_d[:, l0 * D : l1 * D])
            nc.sync.dma_start(
                ka_all[:, l0 * C2 : l1 * C2], ka_d[:, l0 * C2 : l1 * C2]
            )

        _loads(LA, 7)
        _loads(7, L)

        # ---- prelude: rinv for all layers (batched, bit-exact per element),
        # then all row-broadcasts via 3 block-diagonal matmuls ----
        ss_all = const.tile([P_N, L], f32, tag="ss")
        y1_all = const.tile([P_N, L], f32, tag="y1")

        def _rinv_batch(l0, l1):
            n = l1 - l0
            sl = (slice(None), slice(l0, l1))
            sq = small.tile([P_N, n], f32, tag=f"sq{l0}")
            nc.scalar.activation(sq[:], ss_all[sl], AF.Sqrt)
            y0 = small.tile([P_N, n], f32, tag=f"y0{l0}")
            nc.vector.reciprocal(y0[:], sq[:])
            t1 = small.tile([P_N, n], f32, tag=f"t1{l0}")
            nc.vector.tensor_tensor(t1[:], y0[:], y0[:], op=OP.mult)
            nc.vector.tensor_tensor(t1[:], t1[:], ss_all[sl], op=OP.mult)
            nc.vector.tensor_scalar(t1[:], t1[:], -0.5, 1.5, OP.mult, OP.add)
            nc.vector.tensor_tensor(y1_all[sl], t1[:], y0[:], op=OP.mult)

        def _rb_batch(l0, l1):
            # rb = ones.T @ block-diag(rinv): exact (99 zeros + 1.0 * rinv_p)
            n = l1 - l0
            dg = small.tile([P_N, 4 * P_N], f32, tag="dg")
            for i in range(n):
                nc.vector.tensor_scalar_mul(
                    dg[:, i * P_N : (i + 1) * P_N],
                    ident[:P_N, :P_N],
                    y1_all[:, l0 + i : l0 + i + 1],
                )
            rbp = ps_rb.tile([128, 4 * P_N], f32, tag="rbp")
            nc.tensor.matmul(
                rbp[:, : n * P_N], ones_col[:], dg[:, : n * P_N], start=True, stop=True
            )
            nc.scalar.copy(
                rb_all[:, l0 * P_N : l1 * P_N], rbp[:, : n * P_N]
            )

        def _rinv_group(l0, l1):
            # Squares (exact per-layer accumulate) then the batched chain.
            # Emitted just before the first layer that needs it, so queue
            # FIFO order never blocks earlier layers on later loads.
            for l in range(l0, l1):
                scr = scrp.tile([P_N, D], f32, tag="scr")
                nc.scalar.activation(
                    scr[:],
                    kn_all[:, l * D : (l + 1) * D],
                    AF.Square,
                    accum_out=ss_all[:, l : l + 1],
                )
            _rinv_batch(l0, l1)
            for g in range(l0, l1, 4):
                _rb_batch(g, min(g + 4, l1))

        # ---- phase 1+2 interleaved: per-layer selection + streaming out ----
        for l in range(L):
            if l == 0:
                _rinv_group(0, LA)
            elif l == LA:
                _rinv_group(LA, 7)
            elif l == 7:
                _rinv_group(7, L)
            # nkt = K^T * rinv (columns scaled): identical f32 products to
            # normalizing K rows and transposing.
            nkt = nktp.tile([128, C], f32, tag="nkt")
            for j in range(NCH):
                nc.vector.tensor_tensor(
                    nkt[:, j * P_N : (j + 1) * P_N],
                    kt_all[:, l * C + j * P_N : l * C + (j + 1) * P_N],
                    rb_all[:, l * P_N : (l + 1) * P_N],
                    op=OP.mult,
                )

            # scores u = x_l @ nkt : psum [128b, 100p]
            pc = ps_pc.tile([BS, P_N], f32, tag="pc")
            for j in range(NCH):
                nc.tensor.matmul(
                    pc[:],
                    xt_all[:, l * D + j * 128 : l * D + (j + 1) * 128],
                    nkt[:, j * P_N : (j + 1) * P_N],
                    start=(j == 0),
                    stop=(j == NCH - 1),
                )
            u = rowp.tile([BS, P_N], f32, tag="u")
            nc.vector.tensor_copy(u[:], pc[:])

            # top-5 threshold via DVE max8 (comparison-only => exact)
            mm8 = small.tile([BS, 8], f32, tag="mm8")
            nc.vector.max(mm8[:], u[:])
            mask = rowp.tile([BS, P_N], f32, tag="mask")
            nc.vector.tensor_scalar(
                mask[:], u[:], mm8[:, TOP_K - 1 : TOP_K], None, OP.is_ge
            )

            # s[p] = <K,A>/(||K||*||A||): one bf16 gram over the interleaved
            # [K|A] blocks; both diagonals extracted in one masked reduce.
            gg = ps_g.tile([P_N, 2 * P_N], f32, tag="gg")
            for j in range(NCH):
                nc.tensor.matmul(
                    gg[:],
                    ka_all[:, l * C2 + j * 2 * P_N + P_N : l * C2 + (j + 1) * 2 * P_N],
                    ka_all[:, l * C2 + j * 2 * P_N : l * C2 + (j + 1) * 2 * P_N],
                    start=(j == 0),
                    stop=(j == NCH - 1),
                )
            dsc = scrp.tile([P_N, 2 * P_N], f32, tag="dscr")
            nc.vector.tensor_tensor(dsc[:], gg[:], ident2[:], op=OP.mult)
            kaa = small.tile([P_N, 2], f32, tag="kaa")
            nc.vector.reduce_sum(
                kaa[:], dsc[:].rearrange("p (t q) -> p t q", t=2), axis=AX.X
            )
            sqa = small.tile([P_N, 1], f32, tag="sqa")
            nc.scalar.activation(sqa[:], kaa[:, 1:2], AF.Sqrt)
            ra = small.tile([P_N, 1], f32, tag="ra")
            nc.vector.reciprocal(ra[:], sqa[:])
            s_t = small.tile([P_N, 1], f32, tag="s_t")
            nc.vector.tensor_tensor(s_t[:], kaa[:, 0:1], ra[:], op=OP.mult)
            nc.vector.tensor_tensor(s_t[:], s_t[:], y1_all[:, l : l + 1], op=OP.mult)

            # W^T = mask^T * s -> wt_all columns
            mt = ps_mt.tile([P_N, BS], f32, tag="mt")
            nc.tensor.transpose(mt[:], mask[:], ident[:])
            nc.vector.tensor_scalar_mul(
                wt_all[:, l * BS : (l + 1) * BS], mt[:], s_t[:]
            )

            # streaming output for this layer, emitted inline so the PE
            # queue interleaves the next layer's selection with these
            # matmuls and never idles long enough to re-throttle (HAM).
            # 2 matmuls share a double-bank psum tile so each PSUM->SBUF
            # cast moves 1024 columns.
            p_sb = ppool.tile([P_N, NF], bf16, tag="p")
            if l < 2:
                # keep the P prefetch off the DMA engines until the small
                # critical head loads have landed
                nc.gpsimd.tensor_copy(p_sb[:1, :1], ka_all[:1, :1])
            nc.gpsimd.dma_start(p_sb[:], p_d[l])
            ob = obuf.tile([BS, NF], bf16, tag="ob")
            for n in range(6):
                po = ps_o.tile([BS, 1024], f32, tag="po")
                for h in range(2):
                    nc.tensor.matmul(
                        po[:, h * 512 : (h + 1) * 512],
                        wt_all[:, l * BS : (l + 1) * BS],
                        p_sb[:, (2 * n + h) * 512 : (2 * n + h + 1) * 512],
                        start=True,
                        stop=True,
                    )
                if n % 3 < 2:
                    nc.scalar.copy(ob[:, n * 1024 : (n + 1) * 1024], po[:])
                else:
                    nc.vector.tensor_copy(ob[:, n * 1024 : (n + 1) * 1024], po[:])
            nc.scalar.dma_start(o_d[l], ob[:])

    nc.compile()
    _CACHE["nc"] = nc
    return nc


def _pack_inputs(x_query, K_all, A_all, P_all):
    import ml_dtypes

    bf = ml_dtypes.bfloat16
    x = np.asarray(x_query, dtype=np.float32)
    K = np.asarray(K_all, dtype=np.float32)
    A = np.asarray(A_all, dtype=np.float32)
    P = np.asarray(P_all, dtype=np.float32)

    # x^T per core: [128dd, (l, j, b)]
    xt = np.ascontiguousarray(
        x.reshape(N_CORES, BS, L, NCH, 128).transpose(0, 4, 2, 3, 1).reshape(
            N_CORES, 128, L * D
        )
    )
    # K^T: [128dd, (l, j, p)] f32
    kt6 = K.reshape(L, P_N, NCH, 128).transpose(3, 0, 2, 1)  # [128, L, 6, 100]
    kt = np.ascontiguousarray(kt6.reshape(128, L * C))
    # interleaved [K^T | A^T] bf16: [128dd, (l, j, [K100 | A100])]
    at6 = A.reshape(L, P_N, NCH, 128).transpose(3, 0, 2, 1)
    ka = np.empty((128, L, NCH, 2, P_N), dtype=np.float32)
    ka[:, :, :, 0, :] = kt6
    ka[:, :, :, 1, :] = at6
    ka = np.ascontiguousarray(ka.reshape(128, L * C2)).astype(bf)
    # K natural: [p, (l, d)]
    kn = np.ascontiguousarray(K.transpose(1, 0, 2).reshape(P_N, L * D))
    pp = np.ascontiguousarray(P.reshape(L, P_N, NF)).astype(bf)
    return xt, kt, ka, kn, pp


def _run(x_query, K_all, A_all, P_all, trace=False, tmpdir=None):
    from concourse.bass_utils import run_bass_kernel_spmd

    xt, kt, ka, kn, pp = _pack_inputs(x_query, K_all, A_all, P_all)
    nc = _build_nc()
    in_maps = [
        {"x": xt[c], "kt": kt, "ka": ka, "kn": kn, "p": pp} for c in range(N_CORES)
    ]
    br = run_bass_kernel_spmd(
        nc, in_maps, list(range(N_CORES)), trace=trace, tmpdir=tmpdir
    )
    out = np.stack([np.asarray(r["o"]) for r in br.results], axis=0)  # [8, L, BS, NF]
    out = out.astype(np.float32).transpose(1, 0, 2, 3).reshape(L, B, LP, D)
    return out, br


def kernel(x_query, K_all, A_all, P_all):
    out, _ = _run(x_query, K_all, A_all, P_all)
    return out
